# revision 8
# baseline (speedup 1.0000x reference)
"""Trainium2 Bass kernel for the atomic-descriptor builder (radial Chebyshev +
angular Legendre descriptors, N=256 atoms, minimum-image PBC).

Strategy: shard the central-atom axis i across 8 NeuronCores (32 atoms each).
Per core, lay pairs out as [128 j-partitions, 2 j-chunks x 32 atoms free].
The O(N^3) triplet sum is reformulated exactly via the monomial expansion of
the Legendre polynomials: P_l(u_j . u_k) expands into products of symmetric
tensor powers of the unit bond vectors, so

  q_ang[i,n,l] = sum_p c_{l,p} * sum_c w_c * M[i,n,c_p]^2,
  M[i,n,c] = sum_j g[i,j,n] * (u_ij)^{c}   (35 unique components up to deg 4)

which is O(N^2 * 35).  sqrt(w_c) is folded into the tensor-power components,
so squared moments already carry the multiplicity weights.  The per-(i,n)
moments reduce over j (partitions) with small PE matmuls into PSUM (8 atoms
packed per bank along the free axis); the radial sums q_r use a ones-vector
matmul.
"""
import numpy as np
from math import factorial, sqrt

N_ATOMS = 256
NCORES = 8
NI = N_ATOMS // NCORES        # 32 central atoms per core
NCHUNK = 2                    # j-chunks of 128 partitions
W = NCHUNK * NI               # 64 free columns per (chunk, atom)
NFEAT = 9                     # radial features (K_RADIAL+1)
NA = 4                        # angular radial features
RC = 5.0
NCOMP = 35
BLOCKS = [(0, 1), (1, 4), (4, 10), (10, 20), (20, 35)]   # per-degree ranges

# symmetric tensor-power build recipes: (name, src_a, src_b); "r" prefix =
# raw (unscaled) scratch component.  Component order defines the 35-axis.
P2 = [("xx", "x", "x"), ("xy", "x", "y"), ("xz", "x", "z"),
      ("yy", "y", "y"), ("yz", "y", "z"), ("zz", "z", "z")]
P3 = [("xxx", "xx", "x"), ("xxy", "xx", "y"), ("xxz", "xx", "z"),
      ("xyy", "yy", "x"), ("xyz", "rxy", "z"), ("xzz", "zz", "x"),
      ("yyy", "yy", "y"), ("yyz", "yy", "z"), ("yzz", "zz", "y"),
      ("zzz", "zz", "z")]
P4 = [("xxxx", "xx", "xx"), ("xxxy", "xx", "rxy"), ("xxxz", "xx", "rxz"),
      ("xxyy", "xx", "yy"), ("xxyz", "xx", "ryz"), ("xxzz", "xx", "zz"),
      ("xyyy", "rxy", "yy"), ("xyyz", "rxy", "ryz"), ("xyzz", "rxz", "ryz"),
      ("xzzz", "rxz", "zz"), ("yyyy", "yy", "yy"), ("yyyz", "yy", "ryz"),
      ("yyzz", "yy", "zz"), ("yzzz", "ryz", "zz"), ("zzzz", "zz", "zz")]
NAMES = ["1", "x", "y", "z"] + [n for lst in (P2, P3, P4) for n, _, _ in lst]
CIDX = {n: k for k, n in enumerate(NAMES)}


def _mult(name):
    m = factorial(len(name))
    for ch in "xyz":
        m //= factorial(name.count(ch))
    return m


_compiled = {}


def _build_program(box):
    import concourse.bass as bass
    import concourse.bacc as bacc
    import concourse.tile as tile
    from concourse import mybir

    f32 = mybir.dt.float32
    op = mybir.AluOpType
    act = mybir.ActivationFunctionType
    pi = float(np.pi)

    boxf = np.asarray(box, np.float32)
    diag_box = float(np.abs(boxf - np.diag(np.diag(boxf))).max()) == 0.0

    nc = bacc.Bacc("TRN2", target_bir_lowering=False, debug=False,
                   enable_asserts=False)

    s_all = nc.dram_tensor("s_all", [N_ATOMS, 3], f32, kind="ExternalInput")
    si2 = nc.dram_tensor("si2", [1, 3 * W], f32, kind="ExternalInput")
    maskd = nc.dram_tensor("mask", [128, W], f32, kind="ExternalInput")
    outd = nc.dram_tensor("out", [NI, NFEAT + NA * 5], f32,
                          kind="ExternalOutput")

    with tile.TileContext(nc) as tc:
        with tc.tile_pool(name="sb", bufs=1) as sb, \
             tc.tile_pool(name="ps", bufs=1, space="PSUM") as ps:

            def t(shape, tag):
                return sb.tile(shape, f32, tag=tag, name=tag)

            # ---- constant / input loads -------------------------------
            si = t([128, 3, W], "si")          # central-atom fractional coords
            si_src = si2.ap()
            nc.sync.dma_start(out=si[:, :, :], in_=bass.AP(
                tensor=si_src.tensor, offset=si_src.offset,
                ap=[[0, 128]] + list(si_src.ap[1:])))
            mask = t([128, W], "mask")         # 0 at j==i, else 1
            nc.sync.dma_start(out=mask[:, :], in_=maskd.ap())
            sj = []                            # neighbour coords per chunk
            for c in range(NCHUNK):
                sjc = t([128, 3], f"sj{c}")
                nc.sync.dma_start(out=sjc[:, :],
                                  in_=s_all[c * 128:(c + 1) * 128, :])
                sj.append(sjc)
            ones = t([128, 1], "ones")
            nc.vector.memset(ones, 1.0)

            # ---- minimum-image displacements --------------------------
            ds = t([128, 3, W], "ds")
            for c in range(NCHUNK):
                for d in range(3):
                    # ds = s_i - s_j  (central broadcast minus per-partition)
                    nc.vector.tensor_scalar(
                        out=ds[:, d, c * NI:(c + 1) * NI],
                        in0=si[:, d, c * NI:(c + 1) * NI],
                        scalar1=sj[c][:, d:d + 1], scalar2=None,
                        op0=op.subtract)
            A = t([128, 3, W], "wrapA")
            B = t([128, 3, W], "wrapB")
            nc.vector.tensor_scalar(out=A[:, :, :], in0=ds[:, :, :],
                                    scalar1=0.5, scalar2=None, op0=op.is_ge)
            nc.vector.scalar_tensor_tensor(
                out=B[:, :, :], in0=ds[:, :, :], scalar=-0.5, in1=A[:, :, :],
                op0=op.is_le, op1=op.subtract)
            nc.vector.tensor_tensor(out=ds[:, :, :], in0=ds[:, :, :],
                                    in1=B[:, :, :], op=op.add)
            dr = t([128, 3, W], "dr")
            if diag_box:
                for d in range(3):
                    nc.vector.tensor_scalar(
                        out=dr[:, d, :], in0=ds[:, d, :],
                        scalar1=float(boxf[d, d]), scalar2=None, op0=op.mult)
            else:
                for d in range(3):
                    nc.vector.tensor_scalar(
                        out=dr[:, d, :], in0=ds[:, 0, :],
                        scalar1=float(boxf[d, 0]), scalar2=None, op0=op.mult)
                    for e in (1, 2):
                        nc.vector.scalar_tensor_tensor(
                            out=dr[:, d, :], in0=ds[:, e, :],
                            scalar=float(boxf[d, e]), in1=dr[:, d, :],
                            op0=op.mult, op1=op.add)

            # ---- pair distances & unit vectors ------------------------
            dr2 = t([128, 3, W], "dr2")
            nc.vector.tensor_tensor(out=dr2[:, :, :], in0=dr[:, :, :],
                                    in1=dr[:, :, :], op=op.mult)
            rsq = t([128, W], "rsq")
            nc.vector.tensor_tensor(out=rsq[:, :], in0=dr2[:, 0, :],
                                    in1=dr2[:, 1, :], op=op.add)
            nc.vector.scalar_tensor_tensor(
                out=rsq[:, :], in0=dr2[:, 2, :], scalar=1e-12,
                in1=rsq[:, :], op0=op.add, op1=op.add)
            rij = t([128, W], "rij")
            nc.scalar.activation(out=rij[:, :], in_=rsq[:, :], func=act.Sqrt)
            rinv = t([128, W], "rinv")
            nc.vector.reciprocal(out=rinv[:, :], in_=rij[:, :])

            # ---- tensor powers of unit vectors (sqrt(w) folded in) ----
            Tt = t([128, NCOMP, W], "Tt")
            nc.vector.memset(Tt[:, 0, :], 1.0)
            for d in range(3):                 # unit vectors -> comps 1..3
                nc.vector.tensor_tensor(out=Tt[:, 1 + d, :], in0=dr[:, d, :],
                                        in1=rinv[:, :], op=op.mult)
            raw = {"x": Tt[:, 1, :], "y": Tt[:, 2, :], "z": Tt[:, 3, :],
                   "xx": Tt[:, CIDX["xx"], :], "yy": Tt[:, CIDX["yy"], :],
                   "zz": Tt[:, CIDX["zz"], :]}
            for nm in ("rxy", "rxz", "ryz"):
                raw[nm] = t([128, W], nm)
            # p2: diagonal comps (w=1) straight into Tt; off-diagonal raw
            # first, then scaled copy into Tt.
            for nm, a, b in P2:
                dst = Tt[:, CIDX[nm], :] if nm in ("xx", "yy", "zz") \
                    else raw["r" + nm]
                nc.vector.tensor_tensor(out=dst, in0=raw[a], in1=raw[b],
                                        op=op.mult)
            for nm in ("xy", "xz", "yz"):
                nc.vector.tensor_scalar(
                    out=Tt[:, CIDX[nm], :], in0=raw["r" + nm],
                    scalar1=sqrt(2.0), scalar2=None, op0=op.mult)
            for nm, a, b in P3 + P4:
                w = _mult(nm)
                if w == 1:
                    nc.vector.tensor_tensor(out=Tt[:, CIDX[nm], :],
                                            in0=raw[a], in1=raw[b],
                                            op=op.mult)
                else:
                    nc.vector.scalar_tensor_tensor(
                        out=Tt[:, CIDX[nm], :], in0=raw[a],
                        scalar=sqrt(float(w)), in1=raw[b],
                        op0=op.mult, op1=op.mult)

            # ---- radial features (Chebyshev basis, cosine cutoff) -----
            xcl = t([128, W], "xcl")
            nc.vector.tensor_scalar(out=xcl[:, :], in0=rij[:, :],
                                    scalar1=RC, scalar2=None, op0=op.min)
            cosv = t([128, W], "cosv")         # cos(pi*min(r,rc)/rc)
            halfpi = t([128, 1], "halfpi")
            nc.vector.memset(halfpi, pi / 2)
            nc.scalar.activation(out=cosv[:, :], in_=xcl[:, :], func=act.Sin,
                                 scale=-pi / RC, bias=halfpi[:, :])
            step = t([128, W], "step")         # 1 if r < rc (kills LUT tails)
            nc.vector.tensor_scalar(out=step[:, :], in0=rij[:, :],
                                    scalar1=RC, scalar2=None, op0=op.is_lt)
            maskc = t([128, W], "maskc")
            nc.vector.tensor_tensor(out=maskc[:, :], in0=step[:, :],
                                    in1=mask[:, :], op=op.mult)
            phi = t([128, NFEAT, W], "phi")    # masked radial features
            tcos = t([128, W], "tcos")
            nc.vector.scalar_tensor_tensor(
                out=tcos[:, :], in0=cosv[:, :], scalar=1.0, in1=maskc[:, :],
                op0=op.add, op1=op.mult)
            nc.vector.tensor_scalar(out=phi[:, 0, :], in0=tcos[:, :],
                                    scalar1=0.5, scalar2=None, op0=op.mult)
            h = t([128, W], "h")               # 0.5 * fc * mask
            nc.vector.tensor_scalar(out=h[:, :], in0=phi[:, 0, :],
                                    scalar1=0.5, scalar2=None, op0=op.mult)
            b_ = t([128, W], "bche")           # r/rc - 1
            nc.vector.tensor_scalar(out=b_[:, :], in0=rij[:, :],
                                    scalar1=1.0 / RC, scalar2=-1.0,
                                    op0=op.mult, op1=op.add)
            t2 = t([128, W], "t2")
            nc.vector.tensor_tensor(out=t2[:, :], in0=b_[:, :], in1=b_[:, :],
                                    op=op.mult)
            x = t([128, W], "xch")             # Chebyshev argument
            nc.vector.tensor_scalar(out=x[:, :], in0=t2[:, :], scalar1=2.0,
                                    scalar2=-1.0, op0=op.mult, op1=op.add)
            x2 = t([128, W], "x2ch")
            nc.vector.tensor_scalar(out=x2[:, :], in0=x[:, :], scalar1=2.0,
                                    scalar2=None, op0=op.mult)
            nc.vector.scalar_tensor_tensor(      # feature k=1 (T1 = x)
                out=phi[:, 1, :], in0=x[:, :], scalar=1.0, in1=h[:, :],
                op0=op.add, op1=op.mult)
            xx_ = t([128, W], "xxch")
            nc.vector.tensor_tensor(out=xx_[:, :], in0=x[:, :], in1=x[:, :],
                                    op=op.mult)
            Tk = [None, x, t([128, W], "T2")]
            nc.vector.tensor_scalar(out=Tk[2][:, :], in0=xx_[:, :],
                                    scalar1=2.0, scalar2=-1.0,
                                    op0=op.mult, op1=op.add)
            nc.vector.scalar_tensor_tensor(
                out=phi[:, 2, :], in0=Tk[2][:, :], scalar=1.0, in1=h[:, :],
                op0=op.add, op1=op.mult)
            for k in range(3, NFEAT):
                m = t([128, W], f"mch{k}")
                nc.vector.tensor_tensor(out=m[:, :], in0=x2[:, :],
                                        in1=Tk[k - 1][:, :], op=op.mult)
                Tk.append(t([128, W], f"T{k}"))
                nc.vector.tensor_tensor(out=Tk[k][:, :], in0=m[:, :],
                                        in1=Tk[k - 2][:, :], op=op.subtract)
                nc.vector.scalar_tensor_tensor(
                    out=phi[:, k, :], in0=Tk[k][:, :], scalar=1.0,
                    in1=h[:, :], op0=op.add, op1=op.mult)

            # ---- reductions over j (PE matmuls) -----------------------
            psum_qr = ps.tile([1, NFEAT, NI], f32, tag="psqr", name="psqr")
            for c in range(NCHUNK):
                nc.tensor.matmul(psum_qr[:, :, :], ones[:, :],
                                 phi[:, :, c * NI:(c + 1) * NI],
                                 start=(c == 0), stop=(c == NCHUNK - 1))
            # moments: per atom a [4,35] block at PSUM base partition 0,
            # 8 atoms packed per bank along the free axis
            GRP = 8
            pm = [ps.tile([NA, GRP, NCOMP], f32, tag=f"pm{w}", name=f"pm{w}")
                  for w in range(NI // GRP)]
            for i in range(NI):
                wv, il = divmod(i, GRP)
                for c in range(NCHUNK):
                    col = c * NI + i
                    nc.tensor.matmul(pm[wv][:, il, :],
                                     phi[:, 0:NA, col:col + 1],
                                     Tt[:, :, col:col + 1],
                                     start=(c == 0), stop=(c == NCHUNK - 1))

            # ---- squared moments -> angular descriptors ---------------
            Q = t([NA, NI, NCOMP], "Q")        # weighted squared moments
            for wv in range(NI // GRP):
                nc.scalar.activation(out=Q[:, wv * GRP:(wv + 1) * GRP, :],
                                     in_=pm[wv][:, :, :], func=act.Square)
            qang = t([NA, 5, NI], "qang")      # [n, l, i]
            S = t([NA, 3, NI], "S")            # S2, S3, S4
            X = mybir.AxisListType.X
            nc.vector.tensor_reduce(out=qang[:, 0, :], in_=Q[:, :, 0:1],
                                    axis=X, op=op.add)
            nc.vector.tensor_reduce(out=qang[:, 1, :], in_=Q[:, :, 1:4],
                                    axis=X, op=op.add)
            for p in (2, 3, 4):
                lo, hi = BLOCKS[p]
                nc.vector.tensor_reduce(out=S[:, p - 2, :],
                                        in_=Q[:, :, lo:hi], axis=X, op=op.add)
            sc = t([NA, 3, NI], "sc")
            # q2 = 1.5*S2 - 0.5*S0 ; q3 = 2.5*S3 - 1.5*S1
            # q4 = 4.375*S4 - 3.75*S2 + 0.375*S0
            nc.vector.tensor_scalar(out=sc[:, 0, :], in0=qang[:, 0, :],
                                    scalar1=0.5, scalar2=None, op0=op.mult)
            nc.vector.scalar_tensor_tensor(
                out=qang[:, 2, :], in0=S[:, 0, :], scalar=1.5,
                in1=sc[:, 0, :], op0=op.mult, op1=op.subtract)
            nc.vector.tensor_scalar(out=sc[:, 1, :], in0=qang[:, 1, :],
                                    scalar1=1.5, scalar2=None, op0=op.mult)
            nc.vector.scalar_tensor_tensor(
                out=qang[:, 3, :], in0=S[:, 1, :], scalar=2.5,
                in1=sc[:, 1, :], op0=op.mult, op1=op.subtract)
            nc.vector.tensor_scalar(out=sc[:, 2, :], in0=qang[:, 0, :],
                                    scalar1=0.375, scalar2=None, op0=op.mult)
            nc.vector.scalar_tensor_tensor(
                out=sc[:, 2, :], in0=S[:, 0, :], scalar=3.75,
                in1=sc[:, 2, :], op0=op.mult, op1=op.subtract)
            nc.vector.scalar_tensor_tensor(
                out=qang[:, 4, :], in0=S[:, 2, :], scalar=4.375,
                in1=sc[:, 2, :], op0=op.mult, op1=op.subtract)

            # ---- write outputs ----------------------------------------
            qr_sb = t([1, NI, NFEAT], "qr_sb")
            nc.scalar.activation(out=qr_sb[:, :, :],
                                 in_=psum_qr[:, :, :].rearrange("p k i -> p i k"),
                                 func=act.Copy)
            out_ap = outd.ap()
            nc.sync.dma_start(
                out=bass.AP(tensor=out_ap.tensor, offset=out_ap.offset,
                            ap=[[0, 1], [NFEAT + NA * 5, NI], [1, NFEAT]]),
                in_=qr_sb[:, :, :])
            # out[i, 9 + n*5 + l] <- qang[n, l, i]
            nc.sync.dma_start(
                out=bass.AP(tensor=out_ap.tensor,
                            offset=out_ap.offset + NFEAT,
                            ap=[[5, NA], [1, 5], [NFEAT + NA * 5, NI]]),
                in_=qang[:, :, :])

    nc.compile()
    return nc


def _host_prep(R, box):
    R = np.asarray(R, np.float32)
    box = np.asarray(box, np.float32)
    box_inv = np.linalg.inv(box)
    s = np.ascontiguousarray((R @ box_inv.T).astype(np.float32))
    in_maps = []
    for r in range(NCORES):
        sl = s[r * NI:(r + 1) * NI, :]                    # [NI, 3]
        si2 = np.empty((1, 3 * W), np.float32)
        for d in range(3):
            for c in range(NCHUNK):
                si2[0, d * W + c * NI:(d * W + (c + 1) * NI)] = sl[:, d]
        mask = np.ones((128, W), np.float32)
        for i in range(NI):
            g = r * NI + i
            c, j = divmod(g, 128)
            mask[j, c * NI + i] = 0.0
        in_maps.append({"s_all": s, "si2": si2, "mask": mask})
    return in_maps


def kernel(R, box):
    R = np.asarray(R)
    box = np.asarray(box)
    key = np.asarray(box, np.float32).tobytes()
    nc = _compiled.get(key)
    if nc is None:
        nc = _build_program(box)
        _compiled[key] = nc
    in_maps = _host_prep(R, box)
    from concourse.bass_utils import run_bass_kernel_spmd
    res = run_bass_kernel_spmd(nc, in_maps, core_ids=list(range(NCORES)))
    return np.concatenate([res.results[r]["out"] for r in range(NCORES)],
                          axis=0).astype(np.float32)


# revision 41
# speedup vs baseline: 1.6843x; 1.6843x over previous
"""Trainium2 Bass kernel for the atomic-descriptor builder (radial Chebyshev +
angular Legendre descriptors, N=256 atoms, minimum-image PBC).

Strategy: shard the central-atom axis i across 8 NeuronCores (32 atoms each).
Per core, lay pairs out as [128 j-partitions, 2 j-chunks x 32 atoms free].
The O(N^3) triplet sum is reformulated exactly via the monomial expansion of
the Legendre polynomials: P_l(u_j . u_k) expands into products of symmetric
tensor powers of the unit bond vectors u_ij = dr_ij/r_ij:

  q_ang[i,n,l] = sum_c A[c,l] * M[i,n,c]^2,
  M[i,n,c] = sum_j g[i,j,n] * sqrt(w_c) * (u_ij)^{c}   (35 components, deg<=4)

which is O(N^2 * 35).  Per atom, one PE matmul per j-chunk computes
M = Tt_i^T @ phi_i ([35 comps x 9 feats]; row c=0 is the plain radial sum
q_r since component 0 is the constant 1).  A second tiny matmul with the
constant coefficient matrix A folds the squared moments straight into the
5 Legendre channels.  The component axis is ordered so the tensor powers
build as cyclic-rotation trios: each wide [128,3,64] DVE op produces three
components at once, with rotated operand views taken from cyclically
extended tiles (filled by idle-ACT copies).
"""
import numpy as np
from math import sqrt

N_ATOMS = 256
NCORES = 8
NI = N_ATOMS // NCORES        # 32 central atoms per core
NCHUNK = 2                    # j-chunks of 128 partitions
W = NCHUNK * NI               # 64 free columns per (chunk, atom)
NFEAT = 9                     # radial features (K_RADIAL+1)
NA = 4                        # angular radial features
RC = 5.0
NCOMP = 35
GRP = 8                       # atoms per PSUM bank
# fused input block columns: si_rep | sj0 | sj1 | mask | A
C_SI, C_SJ, C_MASK, C_A = 0, 3 * W, 3 * W + 6, 3 * W + 6 + W
NCOL = C_A + 5

# component order: trios built by one wide op each (D = diag squares,
# R = off-diag products, rotN = cyclic rotations)
NAMES = ["1", "x", "y", "z",
         "xx", "yy", "zz", "xy", "yz", "xz",
         "xxx", "yyy", "zzz", "xxy", "yyz", "xzz", "xxz", "xyy", "yzz",
         "xyz",
         "xxxx", "yyyy", "zzzz", "xxyy", "yyzz", "xxzz",
         "xxxy", "yyyz", "xzzz", "xxxz", "xyyy", "yzzz",
         "xxyz", "xyyz", "xyzz"]
# Legendre-in-monomial coefficients: q_l = sum_p CLP[l][p] * S_p
CLP = np.array([
    [1.0, 0, 0, 0, 0],
    [0, 1.0, 0, 0, 0],
    [-0.5, 0, 1.5, 0, 0],
    [0, -1.5, 0, 2.5, 0],
    [0.375, 0, -3.75, 0, 4.375],
], dtype=np.float32)


def _amat():
    """[35, 5] matrix: A[c, l] = CLP[l, degree(c)]."""
    deg = [len(n) if n != "1" else 0 for n in NAMES]
    return np.stack([CLP[:, d] for d in deg], axis=0).astype(np.float32)


_compiled = {}


def _build_program(box):
    import concourse.bass as bass
    import concourse.bacc as bacc
    import concourse.tile as tile
    from concourse import mybir

    f32 = mybir.dt.float32
    op = mybir.AluOpType
    act = mybir.ActivationFunctionType
    pi = float(np.pi)

    boxf = np.asarray(box, np.float32)
    diag_box = float(np.abs(boxf - np.diag(np.diag(boxf))).max()) == 0.0
    eq_diag = diag_box and boxf[0, 0] == boxf[1, 1] == boxf[2, 2]

    nc = bacc.Bacc("TRN2", target_bir_lowering=False, debug=False,
                   enable_asserts=False)

    insd = nc.dram_tensor("ins", [128, NCOL], f32, kind="ExternalInput")
    oqrd = nc.dram_tensor("oqr", [NI, NFEAT], f32, kind="ExternalOutput")
    oangd = nc.dram_tensor("oang", [NA * NI, 5], f32, kind="ExternalOutput")

    with tile.TileContext(nc) as tc:
        with tc.tile_pool(name="sb", bufs=1) as sb, \
             tc.tile_pool(name="ps", bufs=1, space="PSUM") as ps:

            def t(shape, tag):
                return sb.tile(shape, f32, tag=tag, name=tag)

            def bcast(ap_slice, n, axis_len):
                # broadcast [128, n] -> [128, n, axis_len] via stride-0 dim
                return bass.AP(tensor=ap_slice.tensor, offset=ap_slice.offset,
                               ap=[ap_slice.ap[0], ap_slice.ap[1],
                                   [0, axis_len]])

            # ---- fused input load (hot part first, rest second) -------
            ins = t([128, NCOL], "ins")
            in_ap = insd.ap()
            nc.sync.dma_start(out=ins[:, 0:C_MASK], in_=in_ap[:, 0:C_MASK])
            nc.sync.dma_start(out=ins[:, C_MASK:], in_=in_ap[:, C_MASK:])
            mask = ins[:, C_MASK:C_MASK + W]
            A_sb = ins[0:NCOMP, C_A:C_A + 5]

            eps_b = t([128, 1], "eps_b")
            nc.vector.memset(eps_b, 1e-12)
            halfpi = t([128, 1], "halfpi")
            nc.vector.memset(halfpi, pi / 2)

            # ---- minimum-image displacements --------------------------
            ds = t([128, 3, W], "ds")
            for c in range(NCHUNK):
                for d in range(3):
                    k = C_SJ + 3 * c + d
                    nc.vector.tensor_scalar(
                        out=ds[:, d, c * NI:(c + 1) * NI],
                        in0=ins[:, C_SI + d * W + c * NI:
                                C_SI + d * W + (c + 1) * NI],
                        scalar1=ins[:, k:k + 1], scalar2=None,
                        op0=op.subtract)
            # wrap = ds - round(ds), via two fused compare ops
            X_ = t([128, 3, W], "wrapX")
            nc.vector.scalar_tensor_tensor(
                out=X_[:, :, :], in0=ds[:, :, :], scalar=0.5, in1=ds[:, :, :],
                op0=op.is_ge, op1=op.subtract)           # (ds>=.5) - ds
            nc.vector.scalar_tensor_tensor(
                out=ds[:, :, :], in0=ds[:, :, :], scalar=-0.5, in1=X_[:, :, :],
                op0=op.is_le, op1=op.subtract)           # (ds<=-.5) - X
            dr = t([128, 3, W], "dr")
            if eq_diag:
                nc.vector.tensor_scalar(
                    out=dr[:, :, :], in0=ds[:, :, :],
                    scalar1=float(boxf[0, 0]), scalar2=None, op0=op.mult)
            elif diag_box:
                for d in range(3):
                    nc.vector.tensor_scalar(
                        out=dr[:, d, :], in0=ds[:, d, :],
                        scalar1=float(boxf[d, d]), scalar2=None, op0=op.mult)
            else:
                for d in range(3):
                    nc.vector.tensor_scalar(
                        out=dr[:, d, :], in0=ds[:, 0, :],
                        scalar1=float(boxf[d, 0]), scalar2=None, op0=op.mult)
                    for e in (1, 2):
                        nc.vector.scalar_tensor_tensor(
                            out=dr[:, d, :], in0=ds[:, e, :],
                            scalar=float(boxf[d, e]), in1=dr[:, d, :],
                            op0=op.mult, op1=op.add)

            # ---- pair distances & unit vectors ------------------------
            dr2 = t([128, 3, W], "dr2")
            nc.vector.tensor_tensor(out=dr2[:, :, :], in0=dr[:, :, :],
                                    in1=dr[:, :, :], op=op.mult)
            rsq = t([128, W], "rsq")
            nc.vector.tensor_reduce(
                out=rsq[:, :], in_=dr2[:, :, :].rearrange("p d w -> p w d"),
                axis=mybir.AxisListType.X, op=op.add)
            rij = t([128, W], "rij")
            nc.scalar.activation(out=rij[:, :], in_=rsq[:, :], func=act.Sqrt,
                                 bias=eps_b[:, :])       # sqrt(rsq + 1e-12)
            rinv = t([128, W], "rinv")
            nc.vector.reciprocal(out=rinv[:, :], in_=rij[:, :])

            # ---- radial features (Chebyshev basis, cosine cutoff) -----
            xcl = t([128, W], "xcl")
            nc.vector.tensor_scalar(out=xcl[:, :], in0=rij[:, :],
                                    scalar1=RC, scalar2=None, op0=op.min)
            cosv = t([128, W], "cosv")         # cos(pi*min(r,rc)/rc)
            nc.scalar.activation(out=cosv[:, :], in_=xcl[:, :], func=act.Sin,
                                 scale=-pi / RC, bias=halfpi[:, :])
            maskc = t([128, W], "maskc")       # (r < rc) * mask
            nc.vector.scalar_tensor_tensor(
                out=maskc[:, :], in0=rij[:, :], scalar=RC, in1=mask,
                op0=op.is_lt, op1=op.mult)
            phi = t([128, NFEAT, W], "phi")    # masked radial features
            tcos = t([128, W], "tcos")
            nc.vector.scalar_tensor_tensor(
                out=tcos[:, :], in0=cosv[:, :], scalar=1.0, in1=maskc[:, :],
                op0=op.add, op1=op.mult)
            h = t([128, W], "h")               # 0.5 * fc * mask
            nc.scalar.activation(out=h[:, :], in_=tcos[:, :],
                                 func=act.Copy, scale=0.25)
            b_ = t([128, W], "bche")           # r/rc - 1
            nc.scalar.activation(out=b_[:, :], in_=rij[:, :],
                                 func=act.Copy, scale=1.0 / RC, bias=-1.0)
            t2 = t([128, W], "t2")
            nc.vector.tensor_tensor(out=t2[:, :], in0=b_[:, :], in1=b_[:, :],
                                    op=op.mult)
            # Chebyshev ladder by product doubling (depth 6):
            # T2=2x^2-1, T3=2x*T2-x, T4=2T2^2-1, T5=2T2T3-x, T6=2T3^2-1,
            # T7=2T3T4-x, T8=2T4^2-1;  Tall[:,k-1,:] = T_k, T_1 = x
            Tall = t([128, 8, W], "Tall")
            x = Tall[:, 0, :]
            nc.vector.tensor_scalar(out=x, in0=t2[:, :], scalar1=2.0,
                                    scalar2=-1.0, op0=op.mult, op1=op.add)
            Tk = [None] + [Tall[:, k - 1, :] for k in range(1, 9)]
            sq = {k: t([128, W], f"sq{k}") for k in (1, 2, 3, 4)}

            def dbl(dst, src):                 # dst = 2*src^2 - 1
                nc.vector.tensor_tensor(out=sq[src][:, :], in0=Tk[src],
                                        in1=Tk[src], op=op.mult)
                nc.vector.tensor_scalar(out=Tk[dst], in0=sq[src][:, :],
                                        scalar1=2.0, scalar2=-1.0,
                                        op0=op.mult, op1=op.add)

            def addm(dst, a, b):               # dst = 2*Ta*Tb - x
                m = t([128, W], f"m{dst}")
                nc.vector.tensor_tensor(out=m[:, :], in0=Tk[a], in1=Tk[b],
                                        op=op.mult)
                nc.vector.scalar_tensor_tensor(
                    out=Tk[dst], in0=m[:, :], scalar=2.0,
                    in1=x, op0=op.mult, op1=op.subtract)

            dbl(2, 1)
            addm(3, 1, 2)
            dbl(4, 2)
            addm(5, 2, 3)
            dbl(6, 3)
            addm(7, 3, 4)
            dbl(8, 4)
            # ---- tensor powers of unit vectors (sqrt(w) folded in) ----
            # trio structure: D=(xx,yy,zz), R=(xy,yz,xz); rotations come
            # from cyclically extended tiles uex=(x,y,z,x,y), Rex.
            Tt = t([128, NCOMP, W], "Tt")
            nc.vector.memset(Tt[:, 0, :], 1.0)
            for d in range(3):                 # u = dr / r
                nc.vector.tensor_tensor(out=Tt[:, 1 + d, :],
                                        in0=dr[:, d, :], in1=rinv[:, :],
                                        op=op.mult)
            uex = t([128, 5, W], "uex")        # (x, y, z, x, y)
            nc.scalar.activation(out=uex[:, 0:3, :], in_=Tt[:, 1:4, :],
                                 func=act.Copy)
            nc.scalar.activation(out=uex[:, 3:5, :], in_=Tt[:, 1:3, :],
                                 func=act.Copy)
            u, urot, urot2 = uex[:, 0:3, :], uex[:, 1:4, :], uex[:, 2:5, :]
            D = Tt[:, 4:7, :]
            nc.vector.tensor_tensor(out=D, in0=u, in1=u, op=op.mult)
            Rex = t([128, 5, W], "Rex")        # (xy, yz, xz, xy, yz)
            R0 = Rex[:, 0:3, :]
            nc.vector.tensor_tensor(out=R0, in0=u, in1=urot, op=op.mult)
            nc.scalar.activation(out=Rex[:, 3:5, :], in_=Rex[:, 0:2, :],
                                 func=act.Copy)
            Rrot, Rrot2 = Rex[:, 1:4, :], Rex[:, 2:5, :]
            Drot = t([128, 3, W], "Drot")      # (yy, zz, xx)
            nc.scalar.activation(out=Drot[:, 0:2, :], in_=Tt[:, 5:7, :],
                                 func=act.Copy)
            nc.scalar.activation(out=Drot[:, 2:3, :], in_=Tt[:, 4:5, :],
                                 func=act.Copy)
            nc.scalar.activation(out=Tt[:, 7:10, :], in_=R0, func=act.Copy,
                                 scale=sqrt(2.0))        # scaled (xy,yz,xz)

            def trio(dst_lo, in0, in1, w):
                out_sl = Tt[:, dst_lo:dst_lo + 3, :]
                if w == 1.0:
                    nc.vector.tensor_tensor(out=out_sl, in0=in0, in1=in1,
                                            op=op.mult)
                else:
                    nc.vector.scalar_tensor_tensor(
                        out=out_sl, in0=in0, scalar=sqrt(w), in1=in1,
                        op0=op.mult, op1=op.mult)

            trio(10, D, u, 1.0)                # xxx, yyy, zzz
            trio(13, D, urot, 3.0)             # xxy, yyz, xzz
            trio(16, D, urot2, 3.0)            # xxz, xyy, yzz
            nc.vector.scalar_tensor_tensor(    # xyz (w=6)
                out=Tt[:, 19, :], in0=Rex[:, 0, :], scalar=sqrt(6.0),
                in1=uex[:, 2, :], op0=op.mult, op1=op.mult)
            trio(20, D, D, 1.0)                # x4, y4, z4
            trio(23, D, Drot[:, :, :], 6.0)    # x2y2, y2z2, x2z2
            trio(26, D, R0, 4.0)               # x3y, y3z, xz3
            trio(29, D, Rrot2, 4.0)            # x3z, xy3, yz3
            trio(32, D, Rrot, 12.0)            # x2yz, xy2z, xyz2

            # finalize phi per j-chunk so chunk-0 matmuls start while the
            # chunk-1 half is still being written
            for c in range(NCHUNK):
                cs = slice(c * NI, (c + 1) * NI)
                nc.scalar.activation(out=phi[:, 0, cs], in_=tcos[:, cs],
                                     func=act.Copy, scale=0.5)
                hc = h[:, cs]
                h_b = bass.AP(tensor=hc.tensor, offset=hc.offset,
                              ap=[hc.ap[0], [0, 8], hc.ap[1]])
                nc.vector.scalar_tensor_tensor(  # phi_k = (T_k + 1) * h
                    out=phi[:, 1:NFEAT, cs], in0=Tall[:, :, cs], scalar=1.0,
                    in1=h_b, op0=op.add, op1=op.mult)

            # ---- reductions over j (PE matmuls) -----------------------
            # per atom: M = Tt_i^T @ phi_i accumulated over both j-chunks
            # via a paired start/stop PSUM group -> [35 comps, 9 feats];
            # row 0 (component "1") is q_r
            pm = [ps.tile([NCOMP, GRP, NFEAT], f32, tag=f"pm{w}",
                          name=f"pm{w}") for w in range(NI // GRP)]
            for i in range(NI):
                wv, il = divmod(i, GRP)
                for c in range(NCHUNK):
                    col = c * NI + i
                    nc.tensor.matmul(pm[wv][:, il, :],
                                     Tt[:, :, col:col + 1],
                                     phi[:, :, col:col + 1],
                                     start=(c == 0), stop=(c == NCHUNK - 1))

            # ---- squared moments -> 5 Legendre channels (per wave) ----
            M2 = t([NCOMP, NI, NA], "M2")
            qr_sb = t([1, NI, NFEAT], "qr_sb")
            qang = t([128, 5], "qang")         # row i*4+n, col l
            qang_ps = ps.tile([128, 5], f32, tag="qang_ps", name="qang_ps")
            oqr_ap = oqrd.ap()
            oang_ap = oangd.ap()
            for wv in range(NI // GRP):
                lo, hi = wv * GRP, (wv + 1) * GRP
                nc.scalar.activation(out=M2[:, lo:hi, :],
                                     in_=pm[wv][:, :, 0:NA], func=act.Square)
                nc.scalar.activation(out=qr_sb[:, lo:hi, :],
                                     in_=pm[wv][0:1, :, :], func=act.Copy)
            # one full-width matmul: stationary M2 [35, 128], moving A
            nc.tensor.matmul(qang_ps[:, :], M2[:, :, :], A_sb,
                             start=True, stop=True)
            nc.scalar.activation(out=qang[:, :], in_=qang_ps[:, :],
                                 func=act.Copy)
            nc.sync.dma_start(out=oang_ap[:, :], in_=qang[:, :])
            nc.sync.dma_start(
                out=bass.AP(tensor=oqr_ap.tensor, offset=oqr_ap.offset,
                            ap=[[0, 1], [NFEAT, NI], [1, NFEAT]]),
                in_=qr_sb[:, :, :])

    nc.compile()
    return nc


def _host_prep(R, box):
    R = np.asarray(R, np.float32)
    box = np.asarray(box, np.float32)
    box_inv = np.linalg.inv(box)
    s = np.ascontiguousarray((R @ box_inv.T).astype(np.float32))
    A = _amat()
    in_maps = []
    for r in range(NCORES):
        sl = s[r * NI:(r + 1) * NI, :]                    # [NI, 3]
        ins = np.zeros((128, NCOL), np.float32)
        for d in range(3):
            for c in range(NCHUNK):
                ins[:, d * W + c * NI:d * W + (c + 1) * NI] = sl[:, d]
        for c in range(NCHUNK):
            ins[:, C_SJ + 3 * c:C_SJ + 3 * (c + 1)] = \
                s[c * 128:(c + 1) * 128, :]
        mask = np.ones((128, W), np.float32)
        for i in range(NI):
            g = r * NI + i
            c, j = divmod(g, 128)
            mask[j, c * NI + i] = 0.0
        ins[:, C_MASK:C_MASK + W] = mask
        ins[0:NCOMP, C_A:C_A + 5] = A
        in_maps.append({"ins": ins})
    return in_maps


def kernel(R, box):
    R = np.asarray(R)
    box = np.asarray(box)
    key = np.asarray(box, np.float32).tobytes()
    nc = _compiled.get(key)
    if nc is None:
        nc = _build_program(box)
        _compiled[key] = nc
    in_maps = _host_prep(R, box)
    from concourse.bass_utils import run_bass_kernel_spmd
    res = run_bass_kernel_spmd(nc, in_maps, core_ids=list(range(NCORES)))
    parts = []
    for r in range(NCORES):
        qr = res.results[r]["oqr"]                       # [NI, 9]
        qa = res.results[r]["oang"].reshape(NI, NA * 5)  # rows i*4+n, col l
        parts.append(np.concatenate([qr, qa], axis=1))
    return np.concatenate(parts, axis=0).astype(np.float32)


# revision 47
# speedup vs baseline: 1.7081x; 1.0142x over previous
"""Trainium2 Bass kernel for the atomic-descriptor builder (radial Chebyshev +
angular Legendre descriptors, N=256 atoms, minimum-image PBC).

Strategy: shard the central-atom axis i across 8 NeuronCores (32 atoms each).
Per core, lay pairs out as [128 j-partitions, 2 j-chunks x 32 atoms free].
The O(N^3) triplet sum is reformulated exactly via the monomial expansion of
the Legendre polynomials: P_l(u_j . u_k) expands into products of symmetric
tensor powers of the unit bond vectors u_ij = dr_ij/r_ij:

  q_ang[i,n,l] = sum_c A[c,l] * M[i,n,c]^2,
  M[i,n,c] = sum_j g[i,j,n] * sqrt(w_c) * (u_ij)^{c}   (35 components, deg<=4)

which is O(N^2 * 35).  Per atom, one PE matmul per j-chunk computes
M = Tt_i^T @ phi_i ([35 comps x 9 feats]; row c=0 is the plain radial sum
q_r since component 0 is the constant 1).  A second tiny matmul with the
constant coefficient matrix A folds the squared moments straight into the
5 Legendre channels.  The component axis is ordered so the tensor powers
build as cyclic-rotation trios: each wide [128,3,64] DVE op produces three
components at once, with rotated operand views taken from cyclically
extended tiles (filled by idle-ACT copies).
"""
import numpy as np
from math import sqrt

N_ATOMS = 256
NCORES = 8
NI = N_ATOMS // NCORES        # 32 central atoms per core
NCHUNK = 2                    # j-chunks of 128 partitions
W = NCHUNK * NI               # 64 free columns per (chunk, atom)
NFEAT = 9                     # radial features (K_RADIAL+1)
NA = 4                        # angular radial features
RC = 5.0
NCOMP = 35
GRP = 8                       # atoms per PSUM bank
# fused input block columns: si_rep | sj0 | sj1 | mask | A
C_SI, C_SJ, C_MASK, C_A = 0, 3 * W, 3 * W + 6, 3 * W + 6 + W
NCOL = C_A + 5

# component order: trios built by one wide op each (D = diag squares,
# R = off-diag products, rotN = cyclic rotations)
NAMES = ["1", "x", "y", "z",
         "xx", "yy", "zz", "xy", "yz", "xz",
         "xxx", "yyy", "zzz", "xxy", "yyz", "xzz", "xxz", "xyy", "yzz",
         "xyz",
         "xxxx", "yyyy", "zzzz", "xxyy", "yyzz", "xxzz",
         "xxxy", "yyyz", "xzzz", "xxxz", "xyyy", "yzzz",
         "xxyz", "xyyz", "xyzz"]
# Legendre-in-monomial coefficients: q_l = sum_p CLP[l][p] * S_p
CLP = np.array([
    [1.0, 0, 0, 0, 0],
    [0, 1.0, 0, 0, 0],
    [-0.5, 0, 1.5, 0, 0],
    [0, -1.5, 0, 2.5, 0],
    [0.375, 0, -3.75, 0, 4.375],
], dtype=np.float32)


def _amat(L=None):
    """[35, 5] matrix: A[c, l] = CLP[l, degree(c)] (x L^2deg for eq-diag
    boxes, compensating the unnormalized u' = u/L on device)."""
    deg = [len(n) if n != "1" else 0 for n in NAMES]
    A = np.stack([CLP[:, d] for d in deg], axis=0).astype(np.float64)
    if L is not None:
        A *= np.array([float(L) ** (2 * d) for d in deg])[:, None]
    return A.astype(np.float32)


_compiled = {}


def _build_program(box):
    import concourse.bass as bass
    import concourse.bacc as bacc
    import concourse.tile as tile
    from concourse import mybir

    f32 = mybir.dt.float32
    op = mybir.AluOpType
    act = mybir.ActivationFunctionType
    pi = float(np.pi)

    boxf = np.asarray(box, np.float32)
    diag_box = float(np.abs(boxf - np.diag(np.diag(boxf))).max()) == 0.0
    eq_diag = diag_box and boxf[0, 0] == boxf[1, 1] == boxf[2, 2]

    nc = bacc.Bacc("TRN2", target_bir_lowering=False, debug=False,
                   enable_asserts=False)

    insd = nc.dram_tensor("ins", [128, NCOL], f32, kind="ExternalInput")
    oqrd = nc.dram_tensor("oqr", [NI, NFEAT], f32, kind="ExternalOutput")
    oangd = nc.dram_tensor("oang", [NA * NI, 5], f32, kind="ExternalOutput")

    with tile.TileContext(nc) as tc:
        with tc.tile_pool(name="sb", bufs=1) as sb, \
             tc.tile_pool(name="ps", bufs=1, space="PSUM") as ps:

            def t(shape, tag):
                return sb.tile(shape, f32, tag=tag, name=tag)

            def bcast(ap_slice, n, axis_len):
                # broadcast [128, n] -> [128, n, axis_len] via stride-0 dim
                return bass.AP(tensor=ap_slice.tensor, offset=ap_slice.offset,
                               ap=[ap_slice.ap[0], ap_slice.ap[1],
                                   [0, axis_len]])

            # ---- fused input load (hot part first, rest second) -------
            ins = t([128, NCOL], "ins")
            in_ap = insd.ap()
            nc.sync.dma_start(out=ins[:, 0:C_MASK], in_=in_ap[:, 0:C_MASK])
            nc.sync.dma_start(out=ins[:, C_MASK:], in_=in_ap[:, C_MASK:])
            mask = ins[:, C_MASK:C_MASK + W]
            A_sb = ins[0:NCOMP, C_A:C_A + 5]

            eps_b = t([128, 1], "eps_b")
            nc.vector.memset(eps_b, 1e-12)
            halfpi = t([128, 1], "halfpi")
            nc.vector.memset(halfpi, pi / 2)

            # ---- minimum-image displacements --------------------------
            ds = t([128, 3, W], "ds")
            for c in range(NCHUNK):
                for d in range(3):
                    k = C_SJ + 3 * c + d
                    nc.vector.tensor_scalar(
                        out=ds[:, d, c * NI:(c + 1) * NI],
                        in0=ins[:, C_SI + d * W + c * NI:
                                C_SI + d * W + (c + 1) * NI],
                        scalar1=ins[:, k:k + 1], scalar2=None,
                        op0=op.subtract)
            # wrap = ds - round(ds), via two fused compare ops
            X_ = t([128, 3, W], "wrapX")
            nc.vector.scalar_tensor_tensor(
                out=X_[:, :, :], in0=ds[:, :, :], scalar=0.5, in1=ds[:, :, :],
                op0=op.is_ge, op1=op.subtract)           # (ds>=.5) - ds
            nc.vector.scalar_tensor_tensor(
                out=ds[:, :, :], in0=ds[:, :, :], scalar=-0.5, in1=X_[:, :, :],
                op0=op.is_le, op1=op.subtract)           # (ds<=-.5) - X
            dr = t([128, 3, W], "dr")
            if eq_diag:
                # u = dsw/|dsw| is scale-invariant: skip the Cartesian
                # scaling; fold L^2 into the Sqrt scale and L into rinv
                dr = ds
            elif diag_box:
                for d in range(3):
                    nc.vector.tensor_scalar(
                        out=dr[:, d, :], in0=ds[:, d, :],
                        scalar1=float(boxf[d, d]), scalar2=None, op0=op.mult)
            else:
                for d in range(3):
                    nc.vector.tensor_scalar(
                        out=dr[:, d, :], in0=ds[:, 0, :],
                        scalar1=float(boxf[d, 0]), scalar2=None, op0=op.mult)
                    for e in (1, 2):
                        nc.vector.scalar_tensor_tensor(
                            out=dr[:, d, :], in0=ds[:, e, :],
                            scalar=float(boxf[d, e]), in1=dr[:, d, :],
                            op0=op.mult, op1=op.add)

            # ---- pair distances & unit vectors ------------------------
            dr2 = t([128, 3, W], "dr2")
            nc.vector.tensor_tensor(out=dr2[:, :, :], in0=dr[:, :, :],
                                    in1=dr[:, :, :], op=op.mult)
            rsq = t([128, W], "rsq")
            nc.vector.tensor_reduce(
                out=rsq[:, :], in_=dr2[:, :, :].rearrange("p d w -> p w d"),
                axis=mybir.AxisListType.X, op=op.add)
            rij = t([128, W], "rij")
            L2 = float(boxf[0, 0]) ** 2 if eq_diag else 1.0
            nc.scalar.activation(out=rij[:, :], in_=rsq[:, :], func=act.Sqrt,
                                 scale=L2, bias=eps_b[:, :])  # sqrt(+1e-12)
            rinv = t([128, W], "rinv")
            nc.vector.reciprocal(out=rinv[:, :], in_=rij[:, :])
            # for eq_diag boxes u' = dsw/r = u/L; the missing L^p per
            # tensor-power degree is folded into the host-built A matrix

            # ---- radial features (Chebyshev basis, cosine cutoff) -----
            xcl = t([128, W], "xcl")
            nc.vector.tensor_scalar(out=xcl[:, :], in0=rij[:, :],
                                    scalar1=RC, scalar2=None, op0=op.min)
            cosv = t([128, W], "cosv")         # cos(pi*min(r,rc)/rc)
            nc.scalar.activation(out=cosv[:, :], in_=xcl[:, :], func=act.Sin,
                                 scale=-pi / RC, bias=halfpi[:, :])
            maskc = t([128, W], "maskc")       # (r < rc) * mask
            nc.vector.scalar_tensor_tensor(
                out=maskc[:, :], in0=rij[:, :], scalar=RC, in1=mask,
                op0=op.is_lt, op1=op.mult)
            phi = t([128, NFEAT, W], "phi")    # masked radial features
            tcos = t([128, W], "tcos")
            nc.vector.scalar_tensor_tensor(
                out=tcos[:, :], in0=cosv[:, :], scalar=1.0, in1=maskc[:, :],
                op0=op.add, op1=op.mult)
            h = t([128, W], "h")               # 0.5 * fc * mask
            nc.scalar.activation(out=h[:, :], in_=tcos[:, :],
                                 func=act.Copy, scale=0.25)
            b_ = t([128, W], "bche")           # r/rc - 1
            nc.scalar.activation(out=b_[:, :], in_=rij[:, :],
                                 func=act.Copy, scale=1.0 / RC, bias=-1.0)
            t2 = t([128, W], "t2")
            nc.vector.tensor_tensor(out=t2[:, :], in0=b_[:, :], in1=b_[:, :],
                                    op=op.mult)
            # Chebyshev ladder by product doubling (depth 6):
            # T2=2x^2-1, T3=2x*T2-x, T4=2T2^2-1, T5=2T2T3-x, T6=2T3^2-1,
            # T7=2T3T4-x, T8=2T4^2-1;  Tall[:,k-1,:] = T_k, T_1 = x
            Tall = t([128, 8, W], "Tall")
            x = Tall[:, 0, :]
            nc.vector.tensor_scalar(out=x, in0=t2[:, :], scalar1=2.0,
                                    scalar2=-1.0, op0=op.mult, op1=op.add)
            Tk = [None] + [Tall[:, k - 1, :] for k in range(1, 9)]
            sq = {k: t([128, W], f"sq{k}") for k in (1, 2, 3, 4)}

            def dbl(dst, src):                 # dst = 2*src^2 - 1
                nc.vector.tensor_tensor(out=sq[src][:, :], in0=Tk[src],
                                        in1=Tk[src], op=op.mult)
                nc.vector.tensor_scalar(out=Tk[dst], in0=sq[src][:, :],
                                        scalar1=2.0, scalar2=-1.0,
                                        op0=op.mult, op1=op.add)

            def addm(dst, a, b):               # dst = 2*Ta*Tb - x
                m = t([128, W], f"m{dst}")
                nc.vector.tensor_tensor(out=m[:, :], in0=Tk[a], in1=Tk[b],
                                        op=op.mult)
                nc.vector.scalar_tensor_tensor(
                    out=Tk[dst], in0=m[:, :], scalar=2.0,
                    in1=x, op0=op.mult, op1=op.subtract)

            dbl(2, 1)
            addm(3, 1, 2)
            dbl(4, 2)
            addm(5, 2, 3)
            dbl(6, 3)
            addm(7, 3, 4)
            dbl(8, 4)
            # ---- tensor powers of unit vectors (sqrt(w) folded in) ----
            # trio structure: D=(xx,yy,zz), R=(xy,yz,xz); rotations come
            # from cyclically extended tiles uex=(x,y,z,x,y), Rex.
            Tt = t([128, NCOMP, W], "Tt")
            nc.vector.memset(Tt[:, 0, :], 1.0)
            for d in range(3):                 # u = dr / r
                nc.vector.tensor_tensor(out=Tt[:, 1 + d, :],
                                        in0=dr[:, d, :], in1=rinv[:, :],
                                        op=op.mult)
            uex = t([128, 5, W], "uex")        # (x, y, z, x, y)
            nc.scalar.activation(out=uex[:, 0:3, :], in_=Tt[:, 1:4, :],
                                 func=act.Copy)
            nc.scalar.activation(out=uex[:, 3:5, :], in_=Tt[:, 1:3, :],
                                 func=act.Copy)
            u, urot, urot2 = uex[:, 0:3, :], uex[:, 1:4, :], uex[:, 2:5, :]
            D = Tt[:, 4:7, :]
            nc.vector.tensor_tensor(out=D, in0=u, in1=u, op=op.mult)
            Rex = t([128, 5, W], "Rex")        # (xy, yz, xz, xy, yz)
            R0 = Rex[:, 0:3, :]
            nc.vector.tensor_tensor(out=R0, in0=u, in1=urot, op=op.mult)
            nc.scalar.activation(out=Rex[:, 3:5, :], in_=Rex[:, 0:2, :],
                                 func=act.Copy)
            Rrot, Rrot2 = Rex[:, 1:4, :], Rex[:, 2:5, :]
            Drot = t([128, 3, W], "Drot")      # (yy, zz, xx)
            nc.scalar.activation(out=Drot[:, 0:2, :], in_=Tt[:, 5:7, :],
                                 func=act.Copy)
            nc.scalar.activation(out=Drot[:, 2:3, :], in_=Tt[:, 4:5, :],
                                 func=act.Copy)
            nc.scalar.activation(out=Tt[:, 7:10, :], in_=R0, func=act.Copy,
                                 scale=sqrt(2.0))        # scaled (xy,yz,xz)

            def trio(dst_lo, in0, in1, w):
                out_sl = Tt[:, dst_lo:dst_lo + 3, :]
                if w == 1.0:
                    nc.vector.tensor_tensor(out=out_sl, in0=in0, in1=in1,
                                            op=op.mult)
                else:
                    nc.vector.scalar_tensor_tensor(
                        out=out_sl, in0=in0, scalar=sqrt(w), in1=in1,
                        op0=op.mult, op1=op.mult)

            trio(10, D, u, 1.0)                # xxx, yyy, zzz
            trio(13, D, urot, 3.0)             # xxy, yyz, xzz
            trio(16, D, urot2, 3.0)            # xxz, xyy, yzz
            nc.vector.scalar_tensor_tensor(    # xyz (w=6)
                out=Tt[:, 19, :], in0=Rex[:, 0, :], scalar=sqrt(6.0),
                in1=uex[:, 2, :], op0=op.mult, op1=op.mult)
            trio(20, D, D, 1.0)                # x4, y4, z4
            trio(23, D, Drot[:, :, :], 6.0)    # x2y2, y2z2, x2z2
            trio(26, D, R0, 4.0)               # x3y, y3z, xz3
            trio(29, D, Rrot2, 4.0)            # x3z, xy3, yz3
            trio(32, D, Rrot, 12.0)            # x2yz, xy2z, xyz2

            # finalize phi per j-chunk so chunk-0 matmuls start while the
            # chunk-1 half is still being written
            for c in range(NCHUNK):
                cs = slice(c * NI, (c + 1) * NI)
                nc.scalar.activation(out=phi[:, 0, cs], in_=tcos[:, cs],
                                     func=act.Copy, scale=0.5)
                hc = h[:, cs]
                h_b = bass.AP(tensor=hc.tensor, offset=hc.offset,
                              ap=[hc.ap[0], [0, 8], hc.ap[1]])
                nc.vector.scalar_tensor_tensor(  # phi_k = (T_k + 1) * h
                    out=phi[:, 1:NFEAT, cs], in0=Tall[:, :, cs], scalar=1.0,
                    in1=h_b, op0=op.add, op1=op.mult)

            # ---- reductions over j (PE matmuls) -----------------------
            # per atom: M = Tt_i^T @ phi_i accumulated over both j-chunks
            # via a paired start/stop PSUM group -> [35 comps, 9 feats];
            # row 0 (component "1") is q_r
            pm = [ps.tile([NCOMP, GRP, NFEAT], f32, tag=f"pm{w}",
                          name=f"pm{w}") for w in range(NI // GRP)]
            for i in range(NI):
                wv, il = divmod(i, GRP)
                for c in range(NCHUNK):
                    col = c * NI + i
                    nc.tensor.matmul(pm[wv][:, il, :],
                                     Tt[:, :, col:col + 1],
                                     phi[:, :, col:col + 1],
                                     start=(c == 0), stop=(c == NCHUNK - 1))

            # ---- squared moments -> 5 Legendre channels (per wave) ----
            M2 = t([NCOMP, NI, NA], "M2")
            qr_sb = t([1, NI, NFEAT], "qr_sb")
            qang = t([128, 5], "qang")         # row i*4+n, col l
            qang_ps = ps.tile([128, 5], f32, tag="qang_ps", name="qang_ps")
            oqr_ap = oqrd.ap()
            oang_ap = oangd.ap()
            for wv in range(NI // GRP):
                lo, hi = wv * GRP, (wv + 1) * GRP
                nc.scalar.activation(out=qr_sb[:, lo:hi, :],
                                     in_=pm[wv][0:1, :, :], func=act.Copy)
                nc.scalar.activation(out=M2[:, lo:hi, :],
                                     in_=pm[wv][:, :, 0:NA], func=act.Square)
            # one full-width matmul: stationary M2 [35, 128], moving A
            nc.tensor.matmul(qang_ps[:, :], M2[:, :, :], A_sb,
                             start=True, stop=True)
            nc.scalar.activation(out=qang[:, :], in_=qang_ps[:, :],
                                 func=act.Copy)
            nc.sync.dma_start(out=oang_ap[:, :], in_=qang[:, :])
            nc.sync.dma_start(
                out=bass.AP(tensor=oqr_ap.tensor, offset=oqr_ap.offset,
                            ap=[[0, 1], [NFEAT, NI], [1, NFEAT]]),
                in_=qr_sb[:, :, :])

    nc.compile()
    return nc


def _host_prep(R, box):
    R = np.asarray(R, np.float32)
    box = np.asarray(box, np.float32)
    box_inv = np.linalg.inv(box)
    s = np.ascontiguousarray((R @ box_inv.T).astype(np.float32))
    diag = np.abs(box - np.diag(np.diag(box))).max() == 0.0
    eq_diag = diag and box[0, 0] == box[1, 1] == box[2, 2]
    A = _amat(float(box[0, 0]) if eq_diag else None)
    in_maps = []
    for r in range(NCORES):
        sl = s[r * NI:(r + 1) * NI, :]                    # [NI, 3]
        ins = np.zeros((128, NCOL), np.float32)
        for d in range(3):
            for c in range(NCHUNK):
                ins[:, d * W + c * NI:d * W + (c + 1) * NI] = sl[:, d]
        for c in range(NCHUNK):
            ins[:, C_SJ + 3 * c:C_SJ + 3 * (c + 1)] = \
                s[c * 128:(c + 1) * 128, :]
        mask = np.ones((128, W), np.float32)
        for i in range(NI):
            g = r * NI + i
            c, j = divmod(g, 128)
            mask[j, c * NI + i] = 0.0
        ins[:, C_MASK:C_MASK + W] = mask
        ins[0:NCOMP, C_A:C_A + 5] = A
        in_maps.append({"ins": ins})
    return in_maps


def kernel(R, box):
    R = np.asarray(R)
    box = np.asarray(box)
    key = np.asarray(box, np.float32).tobytes()
    nc = _compiled.get(key)
    if nc is None:
        nc = _build_program(box)
        _compiled[key] = nc
    in_maps = _host_prep(R, box)
    from concourse.bass_utils import run_bass_kernel_spmd
    res = run_bass_kernel_spmd(nc, in_maps, core_ids=list(range(NCORES)))
    parts = []
    for r in range(NCORES):
        qr = res.results[r]["oqr"]                       # [NI, 9]
        qa = res.results[r]["oang"].reshape(NI, NA * 5)  # rows i*4+n, col l
        parts.append(np.concatenate([qr, qa], axis=1))
    return np.concatenate(parts, axis=0).astype(np.float32)
